# revision 1
# baseline (speedup 1.0000x reference)
"""AttentiveMatch kernel for Trainium2 (8 NeuronCores, data-parallel over batch).

Reference math (per batch):
    pn = l2norm(p); qn = l2norm(q)
    w  = -(pn @ qn^T) / D          # [S,S]
    mv = (w @ q) / S               # [S,D]
    mn = l2norm(mv)
    out = -mean(pn * mn, -1)       # [S]

Device pipeline (scalars folded, sign flips cancel):
    G^T  = q @ p^T                       [S,S]   matmul 1 (PSUM, fp32)
    A^T  = diag(1/|q_j|) G^T             scale fused into PSUM->SBUF copy
    M^T  = q^T A                         [D,S]   matmul 2 (lhsT = q natural)
    dot_i = p_i . M_i = sum_j (1/|q_j|) (G^T)^2[j,i]   (matmul with rq weights)
    ss_i  = |M_i|^2  = sum_d (M^T)^2[d,i]              (matmul with ones)
    out_i = (1/D) dot_i / (|p_i| sqrt(ss_i))

Each core handles 8 batches; inputs shipped as bf16 in natural and
transposed layouts; all accumulation fp32.
"""

import os
import sys

for _p in ("/opt/trn_rl_repo",):
    if _p not in sys.path:
        sys.path.append(_p)

import numpy as np
import ml_dtypes

import concourse.bacc as bacc
import concourse.mybir as mybir
import concourse.tile as tile
from concourse.bass_utils import run_bass_kernel_spmd

B, S, D = 64, 512, 768
NCORES = 8
BP = B // NCORES          # batches per core
ST = S // 128             # s tiles (4)
KT = D // 128             # d tiles (6)
F32 = mybir.dt.float32
F32R = mybir.dt.float32r
BF16 = mybir.dt.bfloat16
AF = mybir.ActivationFunctionType
ALU = mybir.AluOpType

_NC = None

if os.environ.get("KERNEL_LDW_OPT", "0") == "1":
    import concourse.bass_utils as _bu

    _orig_run_command = _bu.run_command

    def _patched_run_command(cmd, **kw):
        cmd = [
            ("--enable-ldw-opt=true" if c == "--enable-ldw-opt=false" else c)
            for c in cmd
        ]
        return _orig_run_command(cmd, **kw)

    _bu.run_command = _patched_run_command


def _build():
    nc = bacc.Bacc("TRN2", target_bir_lowering=False, debug=False, num_devices=NCORES)
    pn_d = nc.dram_tensor("pn", [BP, 128, ST * D], BF16, kind="ExternalInput")
    qn_d = nc.dram_tensor("qn", [BP, 128, ST * D], BF16, kind="ExternalInput")
    pt_d = nc.dram_tensor("pt", [BP, 128, KT * S], BF16, kind="ExternalInput")
    qt_d = nc.dram_tensor("qt", [BP, 128, KT * S], BF16, kind="ExternalInput")
    out_d = nc.dram_tensor("out", [128, BP * ST], F32, kind="ExternalOutput")

    with tile.TileContext(nc) as tc:
        with (
            tc.tile_pool(name="cst", bufs=1) as cst,
            tc.tile_pool(name="inp", bufs=3) as inp,
            tc.tile_pool(name="ats", bufs=2) as ats,
            tc.tile_pool(name="gps", bufs=3, space="PSUM") as gps,
            tc.tile_pool(name="mps", bufs=3, space="PSUM") as mps,
            tc.tile_pool(name="rps", bufs=1, space="PSUM") as rps,
            tc.tile_pool(name="tps", bufs=1, space="PSUM") as tps,
            tc.tile_pool(name="scr", bufs=2) as scr,
            tc.tile_pool(name="st", bufs=2) as st,
            tc.tile_pool(name="res", bufs=1) as res,
        ):
            wd = res.tile([128, BP * ST], F32)
            ones16 = cst.tile([128, 1], BF16)
            nc.gpsimd.memset(ones16[:], 1.0)
            onef = cst.tile([128, 1], F32)
            nc.gpsimd.memset(onef[:], 1.0)

            for b in range(BP):
                # qt via sync ring, pt via scalar ring (parallel HWDGE rings);
                # batch 0 split into chunks so mm1 starts on the first arrivals
                qt_c = []
                pt_c = []
                nch = 3 if b == 0 else 1
                w = (KT // nch) * S
                for c in range(nch):
                    qc = inp.tile([128, w], BF16, tag=f"qt{c}_{nch}")
                    nc.sync.dma_start(qc[:], qt_d[b, :, c * w:(c + 1) * w])
                    pc = inp.tile([128, w], BF16, tag=f"pt{c}_{nch}")
                    if b == 0:
                        nc.scalar.dma_start(pc[:], pt_d[b, :, c * w:(c + 1) * w])
                    else:
                        nc.sync.dma_start(pc[:], pt_d[b, :, c * w:(c + 1) * w])
                    qt_c.append(qc)
                    pt_c.append(pc)
                q_t = inp.tile([128, ST * D], BF16, tag="q")
                nc.gpsimd.dma_start(q_t[:], qn_d[b])
                p_t = inp.tile([128, ST * D], BF16, tag="p")
                nc.gpsimd.dma_start(p_t[:], pn_d[b])
                kw = 2 if b == 0 else KT

                # q row sum-of-squares via ACT Square+accumulate (needed for rq)
                ssq_q = st.tile([128, ST], F32, tag="ssq_q")
                for t in range(ST):
                    sl = slice(t * D, (t + 1) * D)
                    aq = scr.tile([128, D], BF16, tag="aq")
                    nc.scalar.activation(aq[:], q_t[:, sl], AF.Square,
                                         accum_out=ssq_q[:, t:t + 1])
                sq_q = st.tile([128, ST], F32, tag="sq_q")
                nc.scalar.activation(sq_q[:], ssq_q[:], AF.Sqrt)
                rq = st.tile([128, ST], F32, tag="rq")
                nc.vector.reciprocal(rq[:], sq_q[:])
                sqq16 = st.tile([128, ST], BF16, tag="sqq16")
                nc.vector.tensor_copy(sqq16[:], sq_q[:])

                rows = rps.tile([64, 512], F32, tag="rows")
                trn = tps.tile([128, 2 * ST], F32, tag="trn")

                # mm1: G^T[j,i] = sum_d q[j,d] p[i,d]; A^T = rq * G^T;
                # dot_i = sum_j sq_q[j] (A^T)^2[j,i]  (== sum_j rq_j G^2)
                at_tiles = []
                h_tiles = []
                for j in range(ST):
                    g = gps.tile([128, S], F32, tag="g")
                    for k in range(KT):
                        kc, ko = divmod(k, kw)
                        nc.tensor.matmul(
                            g[:],
                            lhsT=qt_c[kc][:, ko * S + j * 128: ko * S + (j + 1) * 128],
                            rhs=pt_c[kc][:, ko * S: (ko + 1) * S],
                            start=(k == 0), stop=(k == KT - 1),
                        )
                    at = ats.tile([128, S], BF16, tag=f"at{j}")
                    nc.scalar.activation(at[:], g[:], AF.Copy, scale=rq[:, j:j + 1])
                    at_tiles.append(at)
                    h = scr.tile([128, S], BF16, tag=f"h{j}")
                    nc.vector.tensor_mul(h[:], at[:], at[:])
                    h_tiles.append(h)
                for j in range(ST):
                    nc.tensor.matmul(
                        rows[0:1, :], lhsT=sqq16[:, j:j + 1], rhs=h_tiles[j][:],
                        start=(j == 0), stop=(j == ST - 1),
                    )

                # mm2: M^T[d,i] = sum_j q[j,d] A^T[j,i]; ss_row += ones^T @ (M^T)^2
                # ACT squares PSUM directly; DVE sums pairs -> 3 ones-matmuls
                s2_pair = []
                for k in range(KT):
                    mt = mps.tile([128, S], F32, tag="mt")
                    for jt in range(ST):
                        nc.tensor.matmul(
                            mt[:],
                            lhsT=q_t[:, jt * D + k * 128: jt * D + (k + 1) * 128],
                            rhs=at_tiles[jt][:],
                            start=(jt == 0), stop=(jt == ST - 1),
                        )
                    ms = scr.tile([128, S], BF16, tag="ms")
                    nc.vector.tensor_copy(ms[:], mt[:])
                    s2 = scr.tile([128, S], BF16, tag=f"s2{k % 2}")
                    nc.vector.tensor_mul(s2[:], ms[:], ms[:])
                    s2_pair.append(s2)
                    if k % 2 == 1:
                        s2s = scr.tile([128, S], BF16, tag="s2s")
                        nc.vector.tensor_add(s2s[:], s2_pair[0][:], s2_pair[1][:])
                        s2_pair = []
                        nc.tensor.matmul(
                            rows[32:33, :], lhsT=ones16[:], rhs=s2s[:],
                            start=(k == 1), stop=(k == KT - 1),
                        )

                # p row sum-of-squares (only needed for the finals -> late)
                ssq_p = st.tile([128, ST], F32, tag="ssq_p")
                for t in range(ST):
                    sl = slice(t * D, (t + 1) * D)
                    ap_ = scr.tile([128, D], BF16, tag="ap")
                    nc.scalar.activation(ap_[:], p_t[:, sl], AF.Square,
                                         accum_out=ssq_p[:, t:t + 1])
                sq_p = st.tile([128, ST], F32, tag="sq_p")
                nc.scalar.activation(sq_p[:], ssq_p[:], AF.Sqrt)
                rp = st.tile([128, ST], F32, tag="rp")
                nc.vector.reciprocal(rp[:], sq_p[:])

                # transpose the two [1,512] rows into [128, ST] columns
                rowsb = st.tile([64, 512], F32, tag="rowsb")
                nc.vector.tensor_copy(rowsb[:], rows[:])
                for c in range(ST):
                    nc.tensor.matmul(
                        trn[:, c:c + 1],
                        lhsT=rowsb[0:1, c * 128:(c + 1) * 128],
                        rhs=onef[0:1, :], start=(c == 0), stop=False,
                    )
                for c in range(ST):
                    nc.tensor.matmul(
                        trn[:, ST + c: ST + c + 1],
                        lhsT=rowsb[32:33, c * 128:(c + 1) * 128],
                        rhs=onef[32:33, :], start=(c == 0), stop=(c == ST - 1),
                    )

                # wd = (1/D) * dot / (sq_p * sqrt(ss));  sqrt(D^2 ss) folds 1/D
                sd = st.tile([128, ST], F32, tag="sd")
                nc.scalar.activation(sd[:], trn[:, ST: 2 * ST], AF.Sqrt,
                                     scale=float(D) * float(D))
                rs = st.tile([128, ST], F32, tag="rs")
                nc.vector.reciprocal(rs[:], sd[:])
                w1 = st.tile([128, ST], F32, tag="w1")
                nc.vector.tensor_mul(w1[:], trn[:, 0:ST], rp[:])
                nc.vector.tensor_mul(wd[:, b * ST: (b + 1) * ST], w1[:], rs[:])

            nc.sync.dma_start(out_d[:], wd[:])
    nc.compile()
    return nc


def _get_nc():
    global _NC
    if _NC is None:
        _NC = _build()
    return _NC


def _prep_inputs(p, q):
    p = np.asarray(p, dtype=np.float32)
    q = np.asarray(q, dtype=np.float32)
    p16 = p.astype(ml_dtypes.bfloat16)
    q16 = q.astype(ml_dtypes.bfloat16)

    # natural: [core, b, part, t*D + d] with s = t*128 + part
    def nat(x):
        return np.ascontiguousarray(
            x.reshape(NCORES, BP, ST, 128, D).transpose(0, 1, 3, 2, 4)
        ).reshape(NCORES, BP, 128, ST * D)

    # transposed: [core, b, part, k*S + i] with d = k*128 + part
    def tr(x):
        return np.ascontiguousarray(
            x.reshape(NCORES, BP, S, KT, 128).transpose(0, 1, 4, 3, 2)
        ).reshape(NCORES, BP, 128, KT * S)

    pn, qn, pt, qt = nat(p16), nat(q16), tr(p16), tr(q16)
    return [
        {"pn": pn[c], "qn": qn[c], "pt": pt[c], "qt": qt[c]}
        for c in range(NCORES)
    ]


def _postprocess(results):
    o = np.stack([np.asarray(r["out"], dtype=np.float32) for r in results])
    # o[c, part, b*ST + t] is out for batch c*BP+b at i = t*128 + part
    o = o.reshape(NCORES, 128, BP, ST).transpose(0, 2, 3, 1).reshape(B, 1, S)
    return np.ascontiguousarray(o)


def _run(inputs, trace=False, **kw):
    nc = _get_nc()
    in_maps = _prep_inputs(inputs["p"], inputs["q"])
    res = run_bass_kernel_spmd(nc, in_maps, list(range(NCORES)), trace=trace, **kw)
    return _postprocess(res.results), res


def kernel(p, q):
    out, _ = _run({"p": p, "q": q})
    return out



# revision 2
# speedup vs baseline: 1.0735x; 1.0735x over previous
"""AttentiveMatch kernel for Trainium2 — v4 (fp8 DoubleRow, woven pipeline).

Reference math (per batch):
    pn = l2norm(p); qn = l2norm(q)
    w  = -(pn @ qn^T) / D
    mv = (w @ q) / S
    out = -mean(pn * l2norm(mv), -1)

Device pipeline (scalars folded, sign flips cancel; C=16, SS=HSC=256):
    G^T[j,i]  = q.p                    mm1: fp8 DoubleRow, fp32 PSUM
    ssq_q     = diag(q-gram)           PE block-gram + amr(gram, ident)
    at        = C*rq_j*G       (fp8)   per-partition scaled PSUM drain
    h2        = (G/HSC)*at     (fp8)   DVE affine_mul_reduce out
    dotrow    = ones^T h2              PE DoubleRow reduce -> PSUM row 0
    M^T[d,i]  = sum_j at q             mm2: fp8 DoubleRow
    s2        = (M/SS)^2       (fp8)   ACT square w/ scale
    ssrow     = ones^T s2              PE DoubleRow reduce -> PSUM row 0
    ssq_p     = diag(p-gram)
    out_i     = dotrow / (sp*sqrt(ssrow)),  sp = SS*D/HSC*|p_i|

Rows are transposed to columns by bouncing through a DRAM scratch.  The
batch stages are software-pipelined two deep with a hand-woven per-engine
emission order; the last batch computes its finals in row space to skip
the bounce latency on the critical tail.
"""

import os
import sys

for _p in ("/opt/trn_rl_repo",):
    if _p not in sys.path:
        sys.path.append(_p)

import numpy as np
import ml_dtypes

import concourse.bacc as bacc
import concourse.mybir as mybir
import concourse.tile as tile
from concourse.bass_utils import run_bass_kernel_spmd

B, S, D = 64, 512, 768
NCORES = 8
BP = B // NCORES          # batches per core
ST = S // 128             # s tiles (4)
KT = D // 128             # d subtiles (6)
KP = KT // 2              # double-row k pairs (3)
JPAIRS = ST // 2          # double-row j pairs (2)
F32 = mybir.dt.float32
BF16 = mybir.dt.bfloat16
F8 = mybir.dt.float8e4
AF = mybir.ActivationFunctionType
ALU = mybir.AluOpType
DR = mybir.MatmulPerfMode.DoubleRow

C_AT = 16.0                 # at = C_AT * rq_j * G
SS = 256.0                  # s2 = (M/SS)^2
HSC = 256.0                 # h2 = (G/HSC)*at
K_SP = (SS * D / HSC) ** 2  # sp = sqrt(ssq_p*K_SP)

_NC = None


def _build():
    nc = bacc.Bacc("TRN2", target_bir_lowering=False, debug=False, num_devices=NCORES)
    qt_d = nc.dram_tensor("qt", [BP, 128, KT, S], F8, kind="ExternalInput")
    pt_d = nc.dram_tensor("pt", [BP, 128, KT, S], F8, kind="ExternalInput")
    qn_d = nc.dram_tensor("qn", [BP, 128, ST, D], F8, kind="ExternalInput")
    id_d = nc.dram_tensor("idm", [128, 128], BF16, kind="ExternalInput")
    out_d = nc.dram_tensor("out", [128, BP * ST], F32, kind="ExternalOutput")
    out2_d = nc.dram_tensor("out2", [1, S], F32, kind="ExternalOutput")
    scr_d = nc.dram_tensor("rowscr", [BP, 2, S], F32, kind="Internal")

    with tile.TileContext(nc) as tc:
        with (
            tc.tile_pool(name="cst", bufs=1) as cst,
            tc.tile_pool(name="inp", bufs=3) as inp,
            tc.tile_pool(name="ats", bufs=2) as ats,
            tc.tile_pool(name="gps", bufs=3, space="PSUM") as gps,
            tc.tile_pool(name="mts", bufs=3, space="PSUM") as mts,
            tc.tile_pool(name="grm", bufs=2, space="PSUM") as grm,
            tc.tile_pool(name="st", bufs=2) as st,
            tc.tile_pool(name="jnk", bufs=1) as jnk,
            tc.tile_pool(name="res", bufs=1) as res,
        ):
            # get batch 0's inputs moving before anything else on the rings;
            # qt in two pieces so the first gram blocks can start sooner
            qpt0 = inp.tile([128, 2, KT, S], F8, tag="qpt", name="qpt0")
            nc.sync.dma_start(qpt0[:, 0, :, 0:256], qt_d[0][:, :, 0:256])
            nc.gpsimd.dma_start(qpt0[:, 1], pt_d[0])
            nc.sync.dma_start(qpt0[:, 0, :, 256:512], qt_d[0][:, :, 256:512])
            qn0 = inp.tile([128, ST, D], F8, tag="qn", name="qn0")
            nc.gpsimd.dma_start(qn0[:], qn_d[0])
            ident = cst.tile([128, 128], BF16)
            nc.sync.dma_start(ident[:], id_d[:, :])
            # DoubleRow ldweights requires >=16 weight columns; row 0 is read
            ones8 = cst.tile([128, 2, 16], F8)
            nc.vector.memset(ones8[:], 1.0)
            wd = res.tile([128, BP * ST], F32)
            junkD = jnk.tile([128, 512], BF16)
            jacc = jnk.tile([128, 1], F32)

            ctx = {}

            def diag(g_t, acc):
                nc.vector.affine_mul_reduce(
                    out=junkD[:, 0:128], accum_out=acc,
                    in0=g_t[:], in1=ident[:], scale=1.0, bias=0.0,
                )

            def gram(src, t, nm, b):
                g_t = grm.tile([128, 128], F32, tag="grm", name=f"{nm}{t}_{b}")
                sl = slice(t * 128, (t + 1) * 128)
                for k in range(KP):
                    nc.tensor.matmul(
                        g_t[:],
                        lhsT=src[:, 2 * k:2 * k + 2, sl],
                        rhs=src[:, 2 * k:2 * k + 2, sl],
                        start=(k == 0), stop=(k == KP - 1), perf_mode=DR,
                    )
                return g_t

            def mm1j(c, j, b):
                g = gps.tile([128, S], F32, tag="g", name=f"g{j}_{b}")
                qt, pt = c["qt"], c["pt"]
                for k in range(KP):
                    nc.tensor.matmul(
                        g[:],
                        lhsT=qt[:, 2 * k:2 * k + 2, j * 128:(j + 1) * 128],
                        rhs=pt[:, 2 * k:2 * k + 2, :],
                        start=(k == 0), stop=(k == KP - 1), perf_mode=DR,
                    )
                c["g"].append(g)

            for it in range(BP + 2):
                b = it          # early batch
                a = it - 1      # late batch
                f = it - 2      # finish batch

                # ---- input DMAs for b
                if 0 < b < BP:
                    c = ctx[b] = {}
                    qpt = inp.tile([128, 2, KT, S], F8, tag="qpt", name=f"qpt{b}")
                    nc.sync.dma_start(qpt[:, 0], qt_d[b])
                    nc.gpsimd.dma_start(qpt[:, 1], pt_d[b])
                    qn = inp.tile([128, ST, D], F8, tag="qn", name=f"qn{b}")
                    nc.gpsimd.dma_start(qn[:], qn_d[b])
                    c["qt"], c["pt"], c["qn"] = qpt[:, 0], qpt[:, 1], qn
                elif b == 0:
                    c = ctx[0] = {}
                    c["qt"], c["pt"], c["qn"] = qpt0[:, 0], qpt0[:, 1], qn0
                if b < BP:
                    cb = ctx[b]
                    cb["g"] = []
                    cb["ssq_q"] = st.tile([128, ST], F32, tag="ssq_q", name=f"sq{b}")
                    cb["ssq_p"] = st.tile([128, ST], F32, tag="ssq_p", name=f"sp{b}")
                ca = ctx.get(a)

                # ---- early(b): grams, diags, mm1, rq chain
                if b < BP:
                    qg = [gram(cb["qt"], 0, "qg", b), gram(cb["qt"], 1, "qg", b)]
                    diag(qg[0], cb["ssq_q"][:, 0:1])
                    diag(qg[1], cb["ssq_q"][:, 1:2])
                    mm1j(cb, 0, b)
                    qg.append(gram(cb["qt"], 2, "qg", b))
                    diag(qg[2], cb["ssq_q"][:, 2:3])
                    mm1j(cb, 1, b)
                    qg.append(gram(cb["qt"], 3, "qg", b))
                    diag(qg[3], cb["ssq_q"][:, 3:4])
                    mm1j(cb, 2, b)
                    mm1j(cb, 3, b)
                    s1 = st.tile([128, ST], F32, tag="s1", name=f"s1{b}")
                    nc.scalar.activation(s1[:], cb["ssq_q"][:], AF.Sqrt,
                                         scale=1.0 / (C_AT * C_AT))
                    crq = st.tile([128, ST], F32, tag="crq", name=f"crq{b}")
                    nc.vector.reciprocal(crq[:], s1[:])
                    cb["crq"] = crq
                    for t in range(ST):
                        pgt = gram(cb["pt"], t, "pg", b)
                        diag(pgt, cb["ssq_p"][:, t:t + 1])

                # ---- late(a): drains of G, reduces, mm2
                if ca is not None:
                    at = ats.tile([128, ST, S], F8, tag="at", name=f"at{a}")
                    h2 = ats.tile([128, ST, S], F8, tag="h2", name=f"h2{a}")
                    for j in range(ST):
                        if j == 0:
                            nc.vector.tensor_scalar_mul(at[:, j, :], ca["g"][j][:],
                                                        ca["crq"][:, j:j + 1])
                        else:
                            nc.scalar.activation(at[:, j, :], ca["g"][j][:], AF.Copy,
                                                 scale=ca["crq"][:, j:j + 1])
                        nc.vector.affine_mul_reduce(
                            out=h2[:, j, :], accum_out=jacc[:], in0=ca["g"][j][:],
                            in1=at[:, j, :], scale=1.0 / HSC, bias=0.0)
                    qn = ca["qn"]
                    last = a == BP - 1
                    # dot row (DoubleRow, dst partitions 0:16; row 0 read)
                    dotrow = mts.tile([128, S], F32, tag="mt", name=f"dot{a}")
                    for u in range(ST // 2):
                        nc.tensor.matmul(
                            dotrow[0:16, :], lhsT=ones8[:],
                            rhs=h2[:, 2 * u:2 * u + 2, :],
                            start=(u == 0), stop=(u == ST // 2 - 1), perf_mode=DR)
                    dsb = st.tile([1, S], F32, tag="dsb", name=f"dsb{a}")
                    nc.vector.tensor_copy(dsb[0:1, :], dotrow[0:1, :])
                    if not last:
                        nc.sync.dma_start(scr_d[a, 0], dsb[0:1, :])
                    else:
                        # prepare rp in row space for the tail shortcut
                        sp7 = st.tile([128, ST], F32, tag="sp7")
                        nc.scalar.activation(sp7[:], ca["ssq_p"][:], AF.Sqrt,
                                             scale=K_SP)
                        rp7 = st.tile([128, ST], F32, tag="rp7")
                        nc.vector.reciprocal(rp7[:], sp7[:])
                        nc.sync.dma_start(
                            scr_d[a, 0].rearrange("(t p) -> p t", p=128), rp7[:, :])
                        rprow = st.tile([1, S], F32, tag="rprow")
                        nc.sync.dma_start(rprow[0:1, :], scr_d[a, 0])

                    # mm2 + fused s2 drains
                    s2 = ats.tile([128, KT, S], F8, tag="s2", name=f"s2{a}")
                    for k in range(KT):
                        mt = mts.tile([128, S], F32, tag="mt", name=f"mt{k}_{a}")
                        for jp in range(JPAIRS):
                            nc.tensor.matmul(
                                mt[:],
                                lhsT=qn[:, 2 * jp:2 * jp + 2, k * 128:(k + 1) * 128],
                                rhs=at[:, 2 * jp:2 * jp + 2, :],
                                start=(jp == 0), stop=(jp == JPAIRS - 1),
                                perf_mode=DR)
                        nc.scalar.activation(s2[:, k, :], mt[:], AF.Square,
                                             scale=1.0 / SS)

                    # ss row (DoubleRow, dst partitions 0:16)
                    ssrow = mts.tile([128, S], F32, tag="mt", name=f"ss{a}")
                    for u in range(KT // 2):
                        nc.tensor.matmul(
                            ssrow[0:16, :], lhsT=ones8[:],
                            rhs=s2[:, 2 * u:2 * u + 2, :],
                            start=(u == 0), stop=(u == KT // 2 - 1), perf_mode=DR)
                    if not last:
                        ssb = st.tile([1, S], F32, tag="ssb", name=f"ssb{a}")
                        nc.vector.tensor_copy(ssb[0:1, :], ssrow[0:1, :])
                        nc.sync.dma_start(scr_d[a, 1], ssb[0:1, :])
                        dsc = st.tile([128, 2, ST], F32, tag="dsc", name=f"dsc{a}")
                        nc.sync.dma_start(
                            dsc[:, :, :],
                            scr_d[a].rearrange("r (t p) -> p r t", p=128))
                        ca["dsc"] = dsc
                    else:
                        # row-space finals for the last batch
                        t2r = st.tile([1, S], F32, tag="t2r")
                        nc.scalar.activation(t2r[0:1, :], ssrow[0:1, :], AF.Sqrt)
                        r2r = st.tile([1, S], F32, tag="r2r")
                        nc.vector.reciprocal(r2r[0:1, :], t2r[0:1, :])
                        wr = st.tile([1, S], F32, tag="wr")
                        nc.vector.tensor_mul(wr[0:1, :], dsb[0:1, :], r2r[0:1, :])
                        orow = st.tile([1, S], F32, tag="orow")
                        nc.vector.tensor_mul(orow[0:1, :], wr[0:1, :], rprow[0:1, :])
                        nc.sync.dma_start(out2_d[0:1, :], orow[0:1, :])

                # ---- finals for f (columns path)
                if 0 <= f < BP - 1:
                    cf = ctx.pop(f)
                    dsc = cf["dsc"]
                    sp = st.tile([128, ST], F32, tag="sp", name=f"spf{f}")
                    nc.scalar.activation(sp[:], cf["ssq_p"][:], AF.Sqrt, scale=K_SP)
                    rp = st.tile([128, ST], F32, tag="rp", name=f"rp{f}")
                    nc.vector.reciprocal(rp[:], sp[:])
                    t2 = st.tile([128, ST], F32, tag="t2", name=f"t2{f}")
                    nc.scalar.activation(t2[:], dsc[:, 1, :], AF.Sqrt)
                    r2 = st.tile([128, ST], F32, tag="r2", name=f"r2{f}")
                    nc.vector.reciprocal(r2[:], t2[:])
                    w1 = st.tile([128, ST], F32, tag="w1", name=f"w1{f}")
                    nc.vector.tensor_mul(w1[:], dsc[:, 0, :], rp[:])
                    nc.vector.tensor_mul(wd[:, f * ST:(f + 1) * ST], w1[:], r2[:])
                    nc.sync.dma_start(out_d[:, f * ST:(f + 1) * ST],
                                      wd[:, f * ST:(f + 1) * ST])
    nc.compile()
    return nc


def _get_nc():
    global _NC
    if _NC is None:
        _NC = _build()
    return _NC


def _prep_inputs(p, q):
    p = np.asarray(p, dtype=np.float32)
    q = np.asarray(q, dtype=np.float32)
    p8 = p.astype(ml_dtypes.float8_e4m3)
    q8 = q.astype(ml_dtypes.float8_e4m3)

    # transposed: [core, b, part, k, s] with d = k*128 + part
    def tr(x):
        return np.ascontiguousarray(
            x.reshape(NCORES, BP, S, KT, 128).transpose(0, 1, 4, 3, 2)
        )

    # natural: [core, b, part, t, d] with s = t*128 + part
    def nat(x):
        return np.ascontiguousarray(
            x.reshape(NCORES, BP, ST, 128, D).transpose(0, 1, 3, 2, 4)
        )

    qtr, ptr, qna = tr(q8), tr(p8), nat(q8)
    idm = np.ascontiguousarray(np.eye(128, dtype=ml_dtypes.bfloat16))
    return [
        {"qt": qtr[c], "pt": ptr[c], "qn": qna[c], "idm": idm}
        for c in range(NCORES)
    ]


def _postprocess(results):
    o = np.stack([np.asarray(r["out"], dtype=np.float32) for r in results])
    # o[c, part, b*ST + t] is out for batch c*BP+b at i = t*128 + part
    o = o.reshape(NCORES, 128, BP, ST).transpose(0, 2, 3, 1)
    # last batch per core arrives as a row [S] with i = t*128 + p
    o2 = np.stack([np.asarray(r["out2"], dtype=np.float32) for r in results])
    o[:, BP - 1, :, :] = o2.reshape(NCORES, ST, 128)
    return np.ascontiguousarray(o.reshape(B, 1, S))


def _run(inputs, trace=False, **kw):
    nc = _get_nc()
    in_maps = _prep_inputs(inputs["p"], inputs["q"])
    res = run_bass_kernel_spmd(nc, in_maps, list(range(NCORES)), trace=trace, **kw)
    return _postprocess(res.results), res


def kernel(p, q):
    out, _ = _run({"p": p, "q": q})
    return out
